# revision 1
# baseline (speedup 1.0000x reference)
"""BinHD (binary hyperdimensional classifier) Trainium2 kernel.

Reference computation:
    enc  = (x @ W >= 0)                          # [B, D] binary
    resp = enc @ (1-hv).T + (1-enc) @ hv.T       # [B, C] Hamming distances

Algebraic reduction used here: with A = 1 - 2*hv (a +/-1 matrix),
    resp[b, c] = sum_d enc[b, d] * A[c, d] + rowsum_hv[c]
so the device only computes ONE binary GEMM for stage 2, and rowsum_hv is
folded in on the host (it depends only on the input hv, not on enc).

Stage 1 is computed *transposed* (encT = W_chunk.T @ xT, D on partitions) so
stage 2 can contract over D without any on-device transposes; x is
pre-transposed per-shard on the host.

Precision: stage-1 GEMM in fp16 (measured sign-flip impact on resp:
max |err| ~ 6 out of absmax ~4320, i.e. ~1.4e-3 relative worst-case).
enc bits and A entries are exact in fp8e4, accumulation is fp32 in PSUM,
so stage 2 is exact given the stage-1 signs.

Sharding: data-parallel over the batch dim B across 8 cores (B=8192 ->
1024 rows/core); W and A replicated.
"""

import numpy as np
import ml_dtypes

import concourse.bass as bass
import concourse.mybir as mybir
import concourse.tile as tile
from concourse.bass_utils import run_bass_kernel_spmd

N_CORES = 8
B, F, D, C = 8192, 1024, 8192, 512
P = 128          # partition dim
NF = 512         # matmul moving free dim (one PSUM bank of fp32)

_F16 = mybir.dt.float16
_F8 = mybir.dt.float8e4
_F32 = mybir.dt.float32

_NP_F16 = np.float16
_NP_F8 = ml_dtypes.float8_e4m3


def _cap_sync_waits(nc):
    """Hoist surplus sem waits onto engine-level EventSemaphore nops.

    The pinned walrus build allows only 1 sync-wait command per
    DMACopy/compute instruction (2 on EventSemaphore); Tile's scheduler
    emits up to 3 (data dep + cross-queue WAW). Moving waits to a
    preceding same-engine wait-nop preserves ordering: the engine blocks
    before issuing the instruction instead of the instruction carrying
    the wait itself.
    """
    for blk in nc.m.functions[0].blocks:
        out = []
        for ins in blk.instructions:
            si = getattr(ins, "sync_info", None)
            if si is not None and si.on_wait:
                limit = 2 if isinstance(ins, mybir.InstEventSemaphore) else 1
                w = list(si.on_wait)
                if len(w) > limit:
                    excess, keep = w[:-limit], w[-limit:]
                    for i in range(0, len(excess), 2):
                        nop = mybir.InstEventSemaphore(
                            name=nc.get_next_instruction_name(),
                            sync_info=mybir.SyncInfo(
                                on_wait=excess[i:i + 2], on_update=[]
                            ),
                        )
                        nop.engine = ins.engine
                        nc.register_instruction(nop)
                        out.append(nop)
                    si.on_wait = keep
            out.append(ins)
        blk.instructions = out
    return nc


def build_nc(bl=B // N_CORES, f=F, d=D, c=C):
    """Build the per-core Bass module.

    Per-core inputs (host-prepared layouts, see kernel()):
      xt : [P, FK*bl]  fp16   xt[p, k*bl + b] = x_shard[b, k*P + p]
      w  : [DM, P, FK*P] fp16 w[m, p, k*P + j] = W[k*P + p, m*P + j]
      at : [P, DM*c]   fp8    at[p, m*c + j] = 1 - 2*hv[j, m*P + p]
    Output:
      out: [BC, P, c]  f32    out[bc, p, j] = sum_d enc[bc*P+p, d]*A[j, d]
    """
    fk = f // P      # F chunks (contraction of stage 1)
    dm = d // P      # D chunks (rows of encT / contraction of stage 2)
    nb = bl // NF    # B column-chunks in stage 1
    bc_n = bl // P   # B partition-chunks in stage 2

    nc = bass.Bass()
    xt_d = nc.dram_tensor("xt", [P, fk * bl], _F16, kind="ExternalInput")
    w_d = nc.dram_tensor("w", [dm, P, fk * P], _F16, kind="ExternalInput")
    at_d = nc.dram_tensor("at", [P, dm * c], _F8, kind="ExternalInput")
    out_d = nc.dram_tensor("out", [bc_n, P, c], _F32, kind="ExternalOutput")

    with tile.TileContext(nc) as tc:
        with (
            tc.tile_pool(name="xt", bufs=1) as xt_pool,
            tc.tile_pool(name="at", bufs=1) as at_pool,
            tc.tile_pool(name="enc", bufs=1) as enc_pool,
            tc.tile_pool(name="w", bufs=6) as w_pool,
            tc.tile_pool(name="res", bufs=2) as res_pool,
            tc.tile_pool(name="ps1", bufs=4, space=bass.MemorySpace.PSUM) as ps1_pool,
            tc.tile_pool(name="ps2", bufs=2, space=bass.MemorySpace.PSUM) as ps2_pool,
        ):
            # per-chunk xt tiles so the first matmul group only waits on
            # the chunks it reads, not one monolithic 2 MiB transfer.
            # xt rides the Sync HWDGE (8 parallel HW queues; measured
            # faster than SWDGE or Activation-HWDGE for this burst
            # despite a ~9us queue start)
            xts = []
            for k in range(fk):
                xk = xt_pool.tile([P, bl], _F16, tag=f"xt{k}")
                nc.sync.dma_start(xk[:], xt_d[:, k * bl:(k + 1) * bl])
                xts.append(xk)
            enc = enc_pool.tile([P, dm * bl], _F8)

            # ---- stage 1: encT chunks [P(D), bl(B)] = sign(W.T @ xT) ----
            for m in range(dm):
                wt = w_pool.tile([P, fk * P], _F16, tag="w")
                # gpsimd (SWDGE) for the W stream: measured best of all
                # routing permutations (incl. HWDGE head-start variants
                # with bufs 6 and 8 — both lose 3-4us in the prologue).
                # SWDGE paces ~79 GB/s vs the PE's 76 GB/s consumption.
                # Slot-release waits over the descriptor budget are
                # handled by _cap_sync_waits.
                nc.gpsimd.dma_start(wt[:], w_d[m])
                for n in range(nb):
                    ps = ps1_pool.tile([P, NF], _F32, tag="ps1")
                    for k in range(fk):
                        nc.tensor.matmul(
                            ps[:],
                            wt[:, k * P:(k + 1) * P],
                            xts[k][:, n * NF: n * NF + NF],
                            start=(k == 0),
                            stop=(k == fk - 1),
                        )
                    nc.vector.tensor_scalar(
                        enc[:, m * bl + n * NF: m * bl + n * NF + NF],
                        ps[:],
                        0.0,
                        scalar2=None,
                        op0=mybir.AluOpType.is_ge,
                    )

            # at (stage-2 input) loads during stage 1
            at = at_pool.tile([P, dm * c], _F8)
            nc.sync.dma_start(at[:], at_d[:])

            # ---- stage 2: out[b, c] = sum_d enc[d, b] * A[c, d] ----
            # fp8 DoubleRow: contract two D-chunks per matmul (values are
            # 0/±1 in fp8e4, fp32 PSUM accumulation -> still exact)
            enc3 = enc[:].rearrange("p (m b) -> p m b", m=dm)
            at3 = at[:].rearrange("p (m c) -> p m c", m=dm)
            for bc in range(bc_n):
                ps2 = ps2_pool.tile([P, c], _F32, tag="ps2")
                for mp in range(dm // 2):
                    nc.tensor.matmul(
                        ps2[:],
                        enc3[:, 2 * mp:2 * mp + 2, bc * P:(bc + 1) * P],
                        at3[:, 2 * mp:2 * mp + 2, :],
                        start=(mp == 0),
                        stop=(mp == dm // 2 - 1),
                        perf_mode=mybir.MatmulPerfMode.DoubleRow,
                    )
                res = res_pool.tile([P, c], _F32, tag="res")
                nc.vector.tensor_copy(res[:], ps2[:])
                nc.sync.dma_start(out_d[bc], res[:])
    return _cap_sync_waits(nc)


def prep_inputs(x, W, classes_hv, n_cores=N_CORES):
    """Host-side shard + layout + dtype prep. Returns (in_maps, rowsum_hv)."""
    b, f = x.shape
    d = W.shape[1]
    c = classes_hv.shape[0]
    bl = b // n_cores
    fk = f // P
    dm = d // P

    # W -> [dm, P, fk*P] fp16: w[m, p, k*P+j] = W[k*P+p, m*P+j]
    wb = W.astype(_NP_F16)
    w_host = np.ascontiguousarray(
        wb.reshape(fk, P, dm, P).transpose(2, 1, 0, 3).reshape(dm, P, fk * P)
    )

    # A = 1 - 2*hv -> at[p, m*c + j] = A[j, m*P + p]
    A = (1.0 - 2.0 * classes_hv).astype(_NP_F8)
    at_host = np.ascontiguousarray(
        A.reshape(c, dm, P).transpose(2, 1, 0).reshape(P, dm * c)
    )

    rowsum_hv = classes_hv.astype(np.float64).sum(axis=1).astype(np.float32)

    in_maps = []
    for i in range(n_cores):
        xs = x[i * bl:(i + 1) * bl].astype(_NP_F16)  # [bl, f]
        # xt[p, k*bl + b] = xs[b, k*P + p]
        xt_host = np.ascontiguousarray(
            xs.reshape(bl, fk, P).transpose(2, 1, 0).reshape(P, fk * bl)
        )
        in_maps.append({"xt": xt_host, "w": w_host, "at": at_host})
    return in_maps, rowsum_hv


_NC_CACHE = {}


def _get_nc():
    if "nc" not in _NC_CACHE:
        _NC_CACHE["nc"] = build_nc()
    return _NC_CACHE["nc"]


def run(x, W, classes_hv, trace=False, **spmd_kwargs):
    """Run on 8 NeuronCores; returns (resp_int32, BassKernelResults)."""
    in_maps, rowsum_hv = prep_inputs(x, W, classes_hv)
    nc = _get_nc()
    bk = run_bass_kernel_spmd(
        nc, in_maps, list(range(N_CORES)), trace=trace, **spmd_kwargs
    )
    bl = B // N_CORES
    resp = np.concatenate(
        [r["out"].reshape(bl, C) for r in bk.results], axis=0
    )  # [B, C] f32, integer-valued
    resp = resp + rowsum_hv[None, :]
    return resp.astype(np.int32), bk


def kernel(x, W, classes_hv):
    resp, _ = run(np.asarray(x), np.asarray(W), np.asarray(classes_hv))
    return resp



# revision 2
# speedup vs baseline: 1.3151x; 1.3151x over previous
"""BinHD (binary hyperdimensional classifier) Trainium2 kernel.

Reference computation:
    enc  = (x @ W >= 0)                          # [B, D] binary
    resp = enc @ (1-hv).T + (1-enc) @ hv.T       # [B, C] Hamming distances

Algebraic reduction used here: with A = 1 - 2*hv (a +/-1 matrix),
    resp[b, c] = sum_d enc[b, d] * A[c, d] + rowsum_hv[c]
so the device only computes ONE binary GEMM for stage 2, and rowsum_hv is
folded in on the host (it depends only on the input hv, not on enc).

Stage 1 is computed *transposed* (encT = W_chunk.T @ xT, D on partitions) so
stage 2 can contract over D without any on-device transposes; x is
pre-transposed per-shard on the host.

Precision: BOTH stages in fp8e4 with DoubleRow (2 contraction rows per
cell -> 2x FLOP rate, measured 216ns per 256x128x512 MM = 155 TF/s).
Stage-1 sign-flip impact measured on the fixed inputs: max |resp err| = 55
out of absmax ~4320 (rel 1.3e-2, gate 2e-2). enc bits and A entries are
exact in fp8e4, accumulation is fp32 in PSUM, so stage 2 is exact given
the stage-1 signs.

Sharding: data-parallel over the batch dim B across 8 cores (B=8192 ->
1024 rows/core); W and A replicated.
"""

import numpy as np
import ml_dtypes

import concourse.bass as bass
import concourse.mybir as mybir
import concourse.tile as tile
from concourse.bass_utils import run_bass_kernel_spmd

N_CORES = 8
B, F, D, C = 8192, 1024, 8192, 512
P = 128          # partition dim
NF = 512         # matmul moving free dim (one PSUM bank of fp32)

_F8 = mybir.dt.float8e4
_F32 = mybir.dt.float32

_NP_F8 = ml_dtypes.float8_e4m3


def _cap_sync_waits(nc):
    """Hoist surplus sem waits onto engine-level EventSemaphore nops.

    The pinned walrus build allows only 1 sync-wait command per
    DMACopy/compute instruction (2 on EventSemaphore); Tile's scheduler
    emits up to 3 (data dep + cross-queue WAW). Moving waits to a
    preceding same-engine wait-nop preserves ordering: the engine blocks
    before issuing the instruction instead of the instruction carrying
    the wait itself.
    """
    for blk in nc.m.functions[0].blocks:
        out = []
        for ins in blk.instructions:
            si = getattr(ins, "sync_info", None)
            if si is not None and si.on_wait:
                limit = 2 if isinstance(ins, mybir.InstEventSemaphore) else 1
                w = list(si.on_wait)
                if len(w) > limit:
                    excess, keep = w[:-limit], w[-limit:]
                    for i in range(0, len(excess), 2):
                        nop = mybir.InstEventSemaphore(
                            name=nc.get_next_instruction_name(),
                            sync_info=mybir.SyncInfo(
                                on_wait=excess[i:i + 2], on_update=[]
                            ),
                        )
                        nop.engine = ins.engine
                        nc.register_instruction(nop)
                        out.append(nop)
                    si.on_wait = keep
            out.append(ins)
        blk.instructions = out
    return nc


def build_nc(bl=B // N_CORES, f=F, d=D, c=C):
    """Build the per-core Bass module.

    Per-core inputs (host-prepared layouts, see kernel()):
      xt : [P, fk*bl]  fp8    xt[p, k*bl + b] = x_shard[b, k*P + p]
      w  : [DM, P, fk*P] fp8  w[m, p, k*P + j] = W[k*P + p, m*P + j]
      at : [P, dm*c]   fp8    at[p, m*c + j] = 1 - 2*hv[j, m*P + p]
    Output:
      out: [BC, P, c]  f32    out[bc, p, j] = sum_d enc[bc*P+p, d]*A[j, d]
    """
    fk = f // P      # F chunks (contraction of stage 1)
    fp = fk // 2     # F chunk PAIRS (DoubleRow contracts 2 chunks per MM)
    dm = d // P      # D chunks (rows of encT / contraction of stage 2)
    nb = bl // NF    # B column-chunks in stage 1
    bc_n = bl // P   # B partition-chunks in stage 2

    nc = bass.Bass()
    xt_d = nc.dram_tensor("xt", [P, fk * bl], _F8, kind="ExternalInput")
    w_d = nc.dram_tensor("w", [dm, P, fk * P], _F8, kind="ExternalInput")
    at_d = nc.dram_tensor("at", [P, dm * c], _F8, kind="ExternalInput")
    out_d = nc.dram_tensor("out", [bc_n, P, c], _F32, kind="ExternalOutput")

    with tile.TileContext(nc) as tc:
        with (
            tc.tile_pool(name="xt", bufs=1) as xt_pool,
            tc.tile_pool(name="at", bufs=1) as at_pool,
            tc.tile_pool(name="enc", bufs=1) as enc_pool,
            tc.tile_pool(name="w", bufs=6) as w_pool,
            tc.tile_pool(name="res", bufs=2) as res_pool,
            tc.tile_pool(name="ps1", bufs=4, space=bass.MemorySpace.PSUM) as ps1_pool,
            tc.tile_pool(name="ps2", bufs=2, space=bass.MemorySpace.PSUM) as ps2_pool,
        ):
            # xt in k-PAIR chunks so each DoubleRow MM reads one [P, 2, bl]
            # view with k-stride = bl, and the first matmul group only
            # waits on the pairs it reads, not one monolithic transfer.
            # xt rides the Sync HWDGE (8 parallel HW queues; measured
            # faster than SWDGE or Activation-HWDGE for this burst
            # despite a ~9us queue start)
            xps = []
            for q in range(fp):
                xq = xt_pool.tile([P, 2 * bl], _F8, tag=f"xt{q}")
                nc.sync.dma_start(xq[:], xt_d[:, 2 * q * bl:(2 * q + 2) * bl])
                xps.append(xq[:].rearrange("p (k b) -> p k b", k=2))
            enc = enc_pool.tile([P, dm * bl], _F8)

            # ---- stage 1: encT chunks [P(D), bl(B)] = sign(W.T @ xT) ----
            # fp8 DoubleRow: contract two F-chunks per matmul.
            for m in range(dm):
                wt = w_pool.tile([P, fk * P], _F8, tag="w")
                # gpsimd (SWDGE) for the W stream: measured best of all
                # routing permutations (incl. HWDGE head-start variants).
                # SWDGE paces ~79 GB/s vs the PE's ~72 GB/s consumption.
                # Slot-release waits over the descriptor budget are
                # handled by _cap_sync_waits.
                nc.gpsimd.dma_start(wt[:], w_d[m])
                wt3 = wt[:].rearrange("p (k j) -> p k j", k=fk)
                for n in range(nb):
                    ps = ps1_pool.tile([P, NF], _F32, tag="ps1")
                    for q in range(fp):
                        nc.tensor.matmul(
                            ps[:],
                            wt3[:, 2 * q:2 * q + 2, :],
                            xps[q][:, :, n * NF: n * NF + NF],
                            start=(q == 0),
                            stop=(q == fp - 1),
                            perf_mode=mybir.MatmulPerfMode.DoubleRow,
                        )
                    nc.vector.tensor_scalar(
                        enc[:, m * bl + n * NF: m * bl + n * NF + NF],
                        ps[:],
                        0.0,
                        scalar2=None,
                        op0=mybir.AluOpType.is_ge,
                    )

            # at (stage-2 input) loads during stage 1
            at = at_pool.tile([P, dm * c], _F8)
            nc.sync.dma_start(at[:], at_d[:])

            # ---- stage 2: out[b, c] = sum_d enc[d, b] * A[c, d] ----
            # fp8 DoubleRow: contract two D-chunks per matmul (values are
            # 0/±1 in fp8e4, fp32 PSUM accumulation -> still exact)
            enc3 = enc[:].rearrange("p (m b) -> p m b", m=dm)
            at3 = at[:].rearrange("p (m c) -> p m c", m=dm)
            for bc in range(bc_n):
                ps2 = ps2_pool.tile([P, c], _F32, tag="ps2")
                for mp in range(dm // 2):
                    nc.tensor.matmul(
                        ps2[:],
                        enc3[:, 2 * mp:2 * mp + 2, bc * P:(bc + 1) * P],
                        at3[:, 2 * mp:2 * mp + 2, :],
                        start=(mp == 0),
                        stop=(mp == dm // 2 - 1),
                        perf_mode=mybir.MatmulPerfMode.DoubleRow,
                    )
                res = res_pool.tile([P, c], _F32, tag="res")
                nc.vector.tensor_copy(res[:], ps2[:])
                nc.sync.dma_start(out_d[bc], res[:])
    return _cap_sync_waits(nc)


def prep_inputs(x, W, classes_hv, n_cores=N_CORES):
    """Host-side shard + layout + dtype prep. Returns (in_maps, rowsum_hv)."""
    b, f = x.shape
    d = W.shape[1]
    c = classes_hv.shape[0]
    bl = b // n_cores
    fk = f // P
    dm = d // P

    # W -> [dm, P, fk*P] fp8: w[m, p, k*P+j] = W[k*P+p, m*P+j]
    wb = W.astype(_NP_F8)
    w_host = np.ascontiguousarray(
        wb.reshape(fk, P, dm, P).transpose(2, 1, 0, 3).reshape(dm, P, fk * P)
    )

    # A = 1 - 2*hv -> at[p, m*c + j] = A[j, m*P + p]
    A = (1.0 - 2.0 * classes_hv).astype(_NP_F8)
    at_host = np.ascontiguousarray(
        A.reshape(c, dm, P).transpose(2, 1, 0).reshape(P, dm * c)
    )

    rowsum_hv = classes_hv.astype(np.float64).sum(axis=1).astype(np.float32)

    in_maps = []
    for i in range(n_cores):
        xs = x[i * bl:(i + 1) * bl].astype(_NP_F8)  # [bl, f]
        # xt[p, k*bl + b] = xs[b, k*P + p]
        xt_host = np.ascontiguousarray(
            xs.reshape(bl, fk, P).transpose(2, 1, 0).reshape(P, fk * bl)
        )
        in_maps.append({"xt": xt_host, "w": w_host, "at": at_host})
    return in_maps, rowsum_hv


_NC_CACHE = {}


def _get_nc():
    if "nc" not in _NC_CACHE:
        _NC_CACHE["nc"] = build_nc()
    return _NC_CACHE["nc"]


def run(x, W, classes_hv, trace=False, **spmd_kwargs):
    """Run on 8 NeuronCores; returns (resp_int32, BassKernelResults)."""
    in_maps, rowsum_hv = prep_inputs(x, W, classes_hv)
    nc = _get_nc()
    bk = run_bass_kernel_spmd(
        nc, in_maps, list(range(N_CORES)), trace=trace, **spmd_kwargs
    )
    bl = B // N_CORES
    resp = np.concatenate(
        [r["out"].reshape(bl, C) for r in bk.results], axis=0
    )  # [B, C] f32, integer-valued
    resp = resp + rowsum_hv[None, :]
    return resp.astype(np.int32), bk


def kernel(x, W, classes_hv):
    resp, _ = run(np.asarray(x), np.asarray(W), np.asarray(classes_hv))
    return resp


# revision 8
# speedup vs baseline: 1.4911x; 1.1338x over previous
"""BinHD (binary hyperdimensional classifier) Trainium2 kernel.

Reference computation:
    enc  = (x @ W >= 0)                          # [B, D] binary
    resp = enc @ (1-hv).T + (1-enc) @ hv.T       # [B, C] Hamming distances

Algebraic reduction used here: with A = 1 - 2*hv (a +/-1 matrix),
    resp[b, c] = sum_d enc[b, d] * A[c, d] + rowsum_hv[c]
so the device only computes ONE binary GEMM for stage 2, and rowsum_hv is
folded in on the host (it depends only on the input hv, not on enc).

Stage 1 is computed *transposed* (encT = W_chunk.T @ xT, D on partitions) so
stage 2 can contract over D without any on-device transposes; x is
pre-transposed per-shard on the host.

Precision: BOTH stages in fp8e4 with DoubleRow (2 contraction rows per
cell -> 2x FLOP rate, measured 216ns per 256x128x512 MM = 155 TF/s).
Stage-1 sign-flip impact measured on the fixed inputs: max |resp err| = 55
out of absmax ~4320 (rel 1.3e-2, gate 2e-2). enc bits and A entries are
exact in fp8e4, accumulation is fp32 in PSUM, so stage 2 is exact given
the stage-1 signs.

Sharding: data-parallel over the batch dim B across 8 cores (B=8192 ->
1024 rows/core); W and A replicated.
"""

import numpy as np
import ml_dtypes

import concourse.bass as bass
import concourse.mybir as mybir
import concourse.tile as tile
from concourse.bass_utils import run_bass_kernel_spmd

N_CORES = 8
B, F, D, C = 8192, 1024, 8192, 512
P = 128          # partition dim
NF = 512         # matmul moving free dim (one PSUM bank of fp32)

_F8 = mybir.dt.float8e4
_F32 = mybir.dt.float32

_NP_F8 = ml_dtypes.float8_e4m3


def _cap_sync_waits(nc):
    """Hoist surplus sem waits onto engine-level EventSemaphore nops.

    The pinned walrus build allows only 1 sync-wait command per
    DMACopy/compute instruction (2 on EventSemaphore); Tile's scheduler
    emits up to 3 (data dep + cross-queue WAW). Moving waits to a
    preceding same-engine wait-nop preserves ordering: the engine blocks
    before issuing the instruction instead of the instruction carrying
    the wait itself.
    """
    for blk in nc.m.functions[0].blocks:
        out = []
        for ins in blk.instructions:
            si = getattr(ins, "sync_info", None)
            if si is not None and si.on_wait:
                limit = 2 if isinstance(ins, mybir.InstEventSemaphore) else 1
                w = list(si.on_wait)
                if len(w) > limit:
                    excess, keep = w[:-limit], w[-limit:]
                    for i in range(0, len(excess), 2):
                        nop = mybir.InstEventSemaphore(
                            name=nc.get_next_instruction_name(),
                            sync_info=mybir.SyncInfo(
                                on_wait=excess[i:i + 2], on_update=[]
                            ),
                        )
                        nop.engine = ins.engine
                        nc.register_instruction(nop)
                        out.append(nop)
                    si.on_wait = keep
            out.append(ins)
        blk.instructions = out
    return nc


def build_nc(bl=B // N_CORES, f=F, d=D, c=C):
    """Build the per-core Bass module.

    Per-core inputs (host-prepared layouts, see kernel()):
      xt : [P, fk*bl]  fp8    xt[p, k*bl + b] = x_shard[b, k*P + p]
      w  : [DM, P, fk*P] fp8  w[m, p, k*P + j] = W[k*P + p, m*P + j]
      at : [P, dm*c]   fp8    at[p, m*c + j] = 1 - 2*hv[j, m*P + p]
    Output:
      out: [BC, P, c]  f32    out[bc, p, j] = sum_d enc[bc*P+p, d]*A[j, d]
    """
    fk = f // P      # F chunks (contraction of stage 1)
    fp = fk // 2     # F chunk PAIRS (DoubleRow contracts 2 chunks per MM)
    dm = d // P      # D chunks (rows of encT / contraction of stage 2)
    nb = bl // NF    # B column-chunks in stage 1
    bc_n = bl // P   # B partition-chunks in stage 2

    nc = bass.Bass()
    xt_d = nc.dram_tensor("xt", [P, fk * bl], _F8, kind="ExternalInput")
    w_d = nc.dram_tensor("w", [dm, P, fk * P], _F8, kind="ExternalInput")
    at_d = nc.dram_tensor("at", [P, dm * c], _F8, kind="ExternalInput")
    out_d = nc.dram_tensor("out", [bc_n, P, c], _F32, kind="ExternalOutput")

    mh = dm // 2     # enc is split into two tiles at this m boundary

    with tile.TileContext(nc) as tc:
        with (
            tc.tile_pool(name="wu", bufs=1) as wu_pool,
            tc.tile_pool(name="xt", bufs=1) as xt_pool,
            tc.tile_pool(name="at", bufs=1) as at_pool,
            tc.tile_pool(name="enc", bufs=1) as enc_pool,
            tc.tile_pool(name="w", bufs=8) as w_pool,
            tc.tile_pool(name="res", bufs=2) as res_pool,
            tc.tile_pool(name="ps1", bufs=4, space=bass.MemorySpace.PSUM) as ps1_pool,
            tc.tile_pool(name="ps2", bufs=4, space=bass.MemorySpace.PSUM) as ps2_pool,
        ):
            # W stream is split across BOTH DMA paths: the SWDGE alone
            # delivers only ~56 GB/s of fp8 W vs the PE's ~62 GB/s
            # consumption (measured), so even m-chunks ride gpsimd/SWDGE
            # and odd m-chunks ride the Sync HWDGE. w[0] is the very
            # first descriptor on the sync queue so the first matmul
            # group is never weight-starved.
            wts = []
            for m in range(dm):
                wt = w_pool.tile([P, fk * P], _F8, tag="w", name=f"wt{m}")
                wts.append(wt)
            nc.sync.dma_start(wts[0][:], w_d[0])

            # xt pair-chunks: each DoubleRow MM reads one [P, 2, bl] view
            # with k-stride = bl. Each pair is DMA'd in two b-halves with
            # all first-halves issued first, so the first matmul group
            # (which reads b 0..NF of all four pairs) starts ~2us sooner.
            xqs = [
                xt_pool.tile([P, 2 * bl], _F8, tag=f"xt{q}", name=f"xq{q}")
                for q in range(fp)
            ]
            for n in range(nb):
                for q in range(fp):
                    for k in range(2):
                        nc.sync.dma_start(
                            xqs[q][:, k * bl + n * NF: k * bl + (n + 1) * NF],
                            xt_d[:, (2 * q + k) * bl + n * NF:
                                 (2 * q + k) * bl + (n + 1) * NF],
                        )
                if n == 0:
                    # w[1] beats the second xt half-batch on the queue:
                    # it's needed after 8 MMs, the n=1 halves after 4.
                    nc.sync.dma_start(wts[1][:], w_d[1])
            xps = [xq[:].rearrange("p (k b) -> p k b", k=2) for xq in xqs]

            # HAM warmup: the PE clock gate opens only after ~3.4us of
            # sustained matmul activity. Junk MMs on a zeroed scratch
            # tile during the DMA prologue span the wait so the first
            # real MMs run at full clock. Results land in a rotating ps1
            # buf that real groups later overwrite (start=True clears).
            wu = wu_pool.tile([P, NF], _F8)
            nc.vector.memset(wu[:], 0.0)
            pw = ps1_pool.tile([P, NF], _F32, tag="ps1")
            for _ in range(34):
                nc.tensor.matmul(pw[:], wu[:, 0:P], wu[:], start=True, stop=True)

            enc_a = enc_pool.tile([P, mh * bl], _F8, tag="enca")
            enc_b = enc_pool.tile([P, mh * bl], _F8, tag="encb")

            # ---- stage 1: encT chunks [P(D), bl(B)] = sign(W.T @ xT) ----
            # fp8 DoubleRow: contract two F-chunks per matmul.
            for m in range(dm):
                wt = wts[m]
                if m > 1:
                    eng = nc.gpsimd if m % 2 == 0 else nc.sync
                    eng.dma_start(wt[:], w_d[m])
                wt3 = wt[:].rearrange("p (k j) -> p k j", k=fk)
                enc = enc_a if m < mh else enc_b
                mo = m if m < mh else m - mh
                for n in range(nb):
                    ps = ps1_pool.tile([P, NF], _F32, tag="ps1")
                    for q in range(fp):
                        nc.tensor.matmul(
                            ps[:],
                            wt3[:, 2 * q:2 * q + 2, :],
                            xps[q][:, :, n * NF: n * NF + NF],
                            start=(q == 0),
                            stop=(q == fp - 1),
                            perf_mode=mybir.MatmulPerfMode.DoubleRow,
                        )
                    nc.vector.tensor_scalar(
                        enc[:, mo * bl + n * NF: mo * bl + n * NF + NF],
                        ps[:],
                        0.0,
                        scalar2=None,
                        op0=mybir.AluOpType.is_ge,
                    )

            # at (stage-2 input) loads during stage 1
            at = at_pool.tile([P, dm * c], _F8)
            nc.sync.dma_start(at[:], at_d[:])

            # ---- stage 2: out[b, c] = sum_d enc[d, b] * A[c, d] ----
            # fp8 DoubleRow: contract two D-chunks per matmul (values are
            # 0/±1 in fp8e4, fp32 PSUM accumulation -> still exact).
            # enc is split (enc_a: m<mh, enc_b: m>=mh) so the first half
            # of each accumulation group only depends on early is_ge ops
            # -- stage 2 starts the moment the last stage-1 MM retires
            # instead of waiting for the vector engine to drain.
            ea3 = enc_a[:].rearrange("p (m b) -> p m b", m=mh)
            eb3 = enc_b[:].rearrange("p (m b) -> p m b", m=mh)
            at3 = at[:].rearrange("p (m c) -> p m c", m=dm)

            def s2_group(ps_ap, bc, c0, c1):
                for mp in range(dm // 2):
                    e3 = ea3 if 2 * mp < mh else eb3
                    mo = 2 * mp if 2 * mp < mh else 2 * mp - mh
                    nc.tensor.matmul(
                        ps_ap,
                        e3[:, mo:mo + 2, bc * P:(bc + 1) * P],
                        at3[:, 2 * mp:2 * mp + 2, c0:c1],
                        start=(mp == 0),
                        stop=(mp == dm // 2 - 1),
                        perf_mode=mybir.MatmulPerfMode.DoubleRow,
                    )

            for bc in range(bc_n):
                ps2 = ps2_pool.tile([P, c], _F32, tag="ps2")
                if bc < bc_n - 1:
                    s2_group(ps2[:], bc, 0, c)
                    res = res_pool.tile([P, c], _F32, tag="res")
                    nc.vector.tensor_copy(res[:], ps2[:])
                    nc.sync.dma_start(out_d[bc], res[:])
                else:
                    # last chunk in two C-halves so the copy+DMA of the
                    # first half overlaps the second half's matmuls --
                    # cuts the post-last-MM tail roughly in half.
                    ch = c // 2
                    for h in range(2):
                        s2_group(ps2[:, h * ch:(h + 1) * ch], bc, h * ch,
                                 (h + 1) * ch)
                        res = res_pool.tile([P, ch], _F32, tag="resh")
                        nc.vector.tensor_copy(res[:], ps2[:, h * ch:(h + 1) * ch])
                        nc.sync.dma_start(
                            out_d[bc][:, h * ch:(h + 1) * ch], res[:]
                        )
    return _cap_sync_waits(nc)


def prep_inputs(x, W, classes_hv, n_cores=N_CORES):
    """Host-side shard + layout + dtype prep. Returns (in_maps, rowsum_hv)."""
    b, f = x.shape
    d = W.shape[1]
    c = classes_hv.shape[0]
    bl = b // n_cores
    fk = f // P
    dm = d // P

    # W -> [dm, P, fk*P] fp8: w[m, p, k*P+j] = W[k*P+p, m*P+j]
    wb = W.astype(_NP_F8)
    w_host = np.ascontiguousarray(
        wb.reshape(fk, P, dm, P).transpose(2, 1, 0, 3).reshape(dm, P, fk * P)
    )

    # A = 1 - 2*hv -> at[p, m*c + j] = A[j, m*P + p]
    A = (1.0 - 2.0 * classes_hv).astype(_NP_F8)
    at_host = np.ascontiguousarray(
        A.reshape(c, dm, P).transpose(2, 1, 0).reshape(P, dm * c)
    )

    rowsum_hv = classes_hv.astype(np.float64).sum(axis=1).astype(np.float32)

    in_maps = []
    for i in range(n_cores):
        xs = x[i * bl:(i + 1) * bl].astype(_NP_F8)  # [bl, f]
        # xt[p, k*bl + b] = xs[b, k*P + p]
        xt_host = np.ascontiguousarray(
            xs.reshape(bl, fk, P).transpose(2, 1, 0).reshape(P, fk * bl)
        )
        in_maps.append({"xt": xt_host, "w": w_host, "at": at_host})
    return in_maps, rowsum_hv


_NC_CACHE = {}


def _get_nc():
    if "nc" not in _NC_CACHE:
        _NC_CACHE["nc"] = build_nc()
    return _NC_CACHE["nc"]


def run(x, W, classes_hv, trace=False, **spmd_kwargs):
    """Run on 8 NeuronCores; returns (resp_int32, BassKernelResults)."""
    in_maps, rowsum_hv = prep_inputs(x, W, classes_hv)
    nc = _get_nc()
    bk = run_bass_kernel_spmd(
        nc, in_maps, list(range(N_CORES)), trace=trace, **spmd_kwargs
    )
    bl = B // N_CORES
    resp = np.concatenate(
        [r["out"].reshape(bl, C) for r in bk.results], axis=0
    )  # [B, C] f32, integer-valued
    resp = resp + rowsum_hv[None, :]
    return resp.astype(np.int32), bk


def kernel(x, W, classes_hv):
    resp, _ = run(np.asarray(x), np.asarray(W), np.asarray(classes_hv))
    return resp


# revision 13
# speedup vs baseline: 1.5779x; 1.0582x over previous
"""BinHD (binary hyperdimensional classifier) Trainium2 kernel.

Reference computation:
    enc  = (x @ W >= 0)                          # [B, D] binary
    resp = enc @ (1-hv).T + (1-enc) @ hv.T       # [B, C] Hamming distances

Algebraic reduction used here: with A = 1 - 2*hv (a +/-1 matrix),
    resp[b, c] = sum_d enc[b, d] * A[c, d] + rowsum_hv[c]
so the device only computes ONE binary GEMM for stage 2, and rowsum_hv is
folded in on the host (it depends only on the input hv, not on enc).

Stage 1 is computed *transposed* (encT = W_chunk.T @ xT, D on partitions) so
stage 2 can contract over D without any on-device transposes; x is
pre-transposed per-shard on the host.

Precision: BOTH stages in fp8e4 with DoubleRow (2 contraction rows per
cell -> 2x FLOP rate, measured 216ns per 256x128x512 MM = 155 TF/s).
Stage-1 sign-flip impact measured on the fixed inputs: max |resp err| = 55
out of absmax ~4320 (rel 1.3e-2, gate 2e-2). enc bits and A entries are
exact in fp8e4, accumulation is fp32 in PSUM, so stage 2 is exact given
the stage-1 signs.

Sharding: data-parallel over the batch dim B across 8 cores (B=8192 ->
1024 rows/core); W and A replicated.
"""

import numpy as np
import ml_dtypes

import concourse.bass as bass
import concourse.mybir as mybir
import concourse.tile as tile
from concourse.bass_utils import run_bass_kernel_spmd

N_CORES = 8
B, F, D, C = 8192, 1024, 8192, 512
P = 128          # partition dim
NF = 512         # matmul moving free dim (one PSUM bank of fp32)

_F8 = mybir.dt.float8e4
_F32 = mybir.dt.float32

_NP_F8 = ml_dtypes.float8_e4m3


def _cap_sync_waits(nc):
    """Hoist surplus sem waits onto engine-level EventSemaphore nops.

    The pinned walrus build allows only 1 sync-wait command per
    DMACopy/compute instruction (2 on EventSemaphore); Tile's scheduler
    emits up to 3 (data dep + cross-queue WAW). Moving waits to a
    preceding same-engine wait-nop preserves ordering: the engine blocks
    before issuing the instruction instead of the instruction carrying
    the wait itself.
    """
    for blk in nc.m.functions[0].blocks:
        out = []
        for ins in blk.instructions:
            si = getattr(ins, "sync_info", None)
            if si is not None and si.on_wait:
                limit = 2 if isinstance(ins, mybir.InstEventSemaphore) else 1
                w = list(si.on_wait)
                if len(w) > limit:
                    excess, keep = w[:-limit], w[-limit:]
                    for i in range(0, len(excess), 2):
                        nop = mybir.InstEventSemaphore(
                            name=nc.get_next_instruction_name(),
                            sync_info=mybir.SyncInfo(
                                on_wait=excess[i:i + 2], on_update=[]
                            ),
                        )
                        nop.engine = ins.engine
                        nc.register_instruction(nop)
                        out.append(nop)
                    si.on_wait = keep
            out.append(ins)
        blk.instructions = out
    return nc


def build_nc(bl=B // N_CORES, f=F, d=D, c=C):
    """Build the per-core Bass module.

    Per-core inputs (host-prepared layouts, see kernel()):
      xt : [P, fk*bl]  fp8    xt[p, k*bl + b] = x_shard[b, k*P + p]
      w  : [DM, P, fk*P] fp8  w[m, p, k*P + j] = W[k*P + p, m*P + j]
      at : [P, dm*c]   fp8    at[p, m*c + j] = 1 - 2*hv[j, m*P + p]
    Output:
      out: [BC, P, c]  f32    out[bc, p, j] = sum_d enc[bc*P+p, d]*A[j, d]
    """
    fk = f // P      # F chunks (contraction of stage 1)
    fp = fk // 2     # F chunk PAIRS (DoubleRow contracts 2 chunks per MM)
    dm = d // P      # D chunks (rows of encT / contraction of stage 2)
    nb = bl // NF    # B column-chunks in stage 1
    bc_n = bl // P   # B partition-chunks in stage 2

    nc = bass.Bass()
    # xt host layout: [P, fp, nb, 2, NF] -- each (pair q, b-half n) is a
    # contiguous 1 KiB row block, so its DMA moves full-width rows
    # (512 B rows measured only ~22 GB/s per HW queue; 1-2 KiB is fast).
    xt_d = nc.dram_tensor("xt", [P, fk * bl], _F8, kind="ExternalInput")
    # w host layout: m-PAIRS with 2 KiB rows: w[mp, p, h*fk*P + k*P + j]
    # holds chunks m = 2*mp+h.
    w_d = nc.dram_tensor("w", [dm // 2, P, 2 * fk * P], _F8, kind="ExternalInput")
    at_d = nc.dram_tensor("at", [P, dm * c], _F8, kind="ExternalInput")
    out_d = nc.dram_tensor("out", [bc_n, P, c], _F32, kind="ExternalOutput")

    mh = dm // 2     # enc is split into two tiles at this m boundary

    # HAM-warmup scratch: a manually-placed SBUF tensor high above the tile
    # pools' bump allocator. Never written -- the warmup matmuls read
    # whatever garbage is there; a pool tile would trip Tile's
    # read-before-write release assert and a memset would wait ~7us for a
    # DVE/GpSimd engine to come up.
    wu_h = nc.alloc_sbuf_tensor_at("wu_scratch", [P, NF], _F8, offset=163840)

    with tile.TileContext(nc) as tc:
        with (
            tc.tile_pool(name="xt", bufs=1) as xt_pool,
            tc.tile_pool(name="at", bufs=1) as at_pool,
            tc.tile_pool(name="enc", bufs=1) as enc_pool,
            tc.tile_pool(name="w", bufs=6) as w_pool,
            tc.tile_pool(name="res", bufs=2) as res_pool,
            tc.tile_pool(name="ps1", bufs=4, space=bass.MemorySpace.PSUM) as ps1_pool,
            tc.tile_pool(name="ps2", bufs=4, space=bass.MemorySpace.PSUM) as ps2_pool,
        ):
            # W stream split across BOTH DMA paths (SWDGE alone delivers
            # ~56 GB/s vs the PE's ~62 GB/s consumption): even pairs ride
            # gpsimd/SWDGE, odd pairs the Sync HWDGE. The first pair is
            # DMA'd as two per-m transfers so the first matmul group only
            # waits on w[0], and it is the first descriptor on the queue.
            wps = []
            for mp2 in range(dm // 2):
                wt = w_pool.tile([P, 2 * fk * P], _F8, tag="w", name=f"wt{mp2}")
                wps.append(wt)
            nc.sync.dma_start(wps[0][:, 0:fk * P], w_d[0][:, 0:fk * P])

            # xt pair-chunks, tile layout [P, (n k b)]: per-(q, n) DMAs of
            # contiguous 1 KiB rows; all n=0 blocks first (the first
            # matmul group reads b 0..NF of all four pairs).
            xqs = [
                xt_pool.tile([P, 2 * bl], _F8, tag=f"xt{q}", name=f"xq{q}")
                for q in range(fp)
            ]
            for q in range(fp):
                nc.sync.dma_start(
                    xqs[q][:, 0:2 * NF],
                    xt_d[:, q * 2 * bl: q * 2 * bl + 2 * NF],
                )
            nc.sync.dma_start(wps[0][:, fk * P:], w_d[0][:, fk * P:])
            for q in range(fp):
                nc.sync.dma_start(
                    xqs[q][:, 2 * NF:4 * NF],
                    xt_d[:, q * 2 * bl + 2 * NF: q * 2 * bl + 4 * NF],
                )
            # view: [p, n, k, b]
            xps = [
                xq[:].rearrange("p (n k b) -> p n k b", n=nb, k=2) for xq in xqs
            ]

            # HAM warmup: the PE clock gate opens only after ~3.4us of
            # sustained matmul activity. Junk MMs on the (uninitialized)
            # scratch during the DMA prologue span the wait so the first
            # real MMs run at full clock; results land in a ps1 buf that
            # real groups later overwrite (start=True clears).
            pw = ps1_pool.tile([P, NF], _F32, tag="ps1")
            for _ in range(40):
                nc.tensor.matmul(pw[:], wu_h[:, 0:P], wu_h[:], start=True, stop=True)

            enc_a = enc_pool.tile([P, mh * bl], _F8, tag="enca")
            enc_b = enc_pool.tile([P, mh * bl], _F8, tag="encb")

            # ---- stage 1: encT chunks [P(D), bl(B)] = sign(W.T @ xT) ----
            # fp8 DoubleRow: contract two F-chunks per matmul.
            for m in range(dm):
                wt = wps[m // 2]
                if m >= 2 and m % 2 == 0:
                    eng = nc.gpsimd if m % 4 == 0 else nc.sync
                    eng.dma_start(wt[:], w_d[m // 2])
                wt4 = wt[:].rearrange("p (h k j) -> p h k j", h=2, k=fk)
                enc = enc_a if m < mh else enc_b
                mo = m if m < mh else m - mh
                for n in range(nb):
                    ps = ps1_pool.tile([P, NF], _F32, tag="ps1")
                    for q in range(fp):
                        nc.tensor.matmul(
                            ps[:],
                            wt4[:, m % 2, 2 * q:2 * q + 2, :],
                            xps[q][:, n, :, :],
                            start=(q == 0),
                            stop=(q == fp - 1),
                            perf_mode=mybir.MatmulPerfMode.DoubleRow,
                        )
                    nc.vector.tensor_scalar(
                        enc[:, mo * bl + n * NF: mo * bl + n * NF + NF],
                        ps[:],
                        0.0,
                        scalar2=None,
                        op0=mybir.AluOpType.is_ge,
                    )

            # at (stage-2 input) loads during stage 1
            at = at_pool.tile([P, dm * c], _F8)
            nc.sync.dma_start(at[:], at_d[:])

            # ---- stage 2: out[b, c] = sum_d enc[d, b] * A[c, d] ----
            # fp8 DoubleRow: contract two D-chunks per matmul (values are
            # 0/±1 in fp8e4, fp32 PSUM accumulation -> still exact).
            # enc is split (enc_a: m<mh, enc_b: m>=mh) so the first half
            # of each accumulation group only depends on early is_ge ops
            # -- stage 2 starts the moment the last stage-1 MM retires
            # instead of waiting for the vector engine to drain.
            ea3 = enc_a[:].rearrange("p (m b) -> p m b", m=mh)
            eb3 = enc_b[:].rearrange("p (m b) -> p m b", m=mh)
            at3 = at[:].rearrange("p (m c) -> p m c", m=dm)

            def s2_group(ps_ap, bc, c0, c1):
                for mp in range(dm // 2):
                    e3 = ea3 if 2 * mp < mh else eb3
                    mo = 2 * mp if 2 * mp < mh else 2 * mp - mh
                    nc.tensor.matmul(
                        ps_ap,
                        e3[:, mo:mo + 2, bc * P:(bc + 1) * P],
                        at3[:, 2 * mp:2 * mp + 2, c0:c1],
                        start=(mp == 0),
                        stop=(mp == dm // 2 - 1),
                        perf_mode=mybir.MatmulPerfMode.DoubleRow,
                    )

            for bc in range(bc_n):
                ps2 = ps2_pool.tile([P, c], _F32, tag="ps2")
                if bc < bc_n - 1:
                    s2_group(ps2[:], bc, 0, c)
                    res = res_pool.tile([P, c], _F32, tag="res")
                    nc.vector.tensor_copy(res[:], ps2[:])
                    nc.sync.dma_start(out_d[bc], res[:])
                else:
                    # last chunk in two C-halves so the copy+DMA of the
                    # first half overlaps the second half's matmuls --
                    # cuts the post-last-MM tail roughly in half.
                    ch = c // 2
                    for h in range(2):
                        s2_group(ps2[:, h * ch:(h + 1) * ch], bc, h * ch,
                                 (h + 1) * ch)
                        res = res_pool.tile([P, ch], _F32, tag="resh")
                        nc.vector.tensor_copy(res[:], ps2[:, h * ch:(h + 1) * ch])
                        nc.sync.dma_start(
                            out_d[bc][:, h * ch:(h + 1) * ch], res[:]
                        )
    return _cap_sync_waits(nc)


def prep_inputs(x, W, classes_hv, n_cores=N_CORES):
    """Host-side shard + layout + dtype prep. Returns (in_maps, rowsum_hv)."""
    b, f = x.shape
    d = W.shape[1]
    c = classes_hv.shape[0]
    bl = b // n_cores
    fk = f // P
    dm = d // P

    # W -> m-pairs with 2 KiB rows: w[mp, p, h*fk*P + k*P + j] =
    # W[k*P+p, (2*mp+h)*P+j]
    wb = W.astype(_NP_F8)
    w_host = np.ascontiguousarray(
        wb.reshape(fk, P, dm // 2, 2, P)
        .transpose(2, 1, 3, 0, 4)
        .reshape(dm // 2, P, 2 * fk * P)
    )

    # A = 1 - 2*hv -> at[p, m*c + j] = A[j, m*P + p]
    A = (1.0 - 2.0 * classes_hv).astype(_NP_F8)
    at_host = np.ascontiguousarray(
        A.reshape(c, dm, P).transpose(2, 1, 0).reshape(P, dm * c)
    )

    rowsum_hv = classes_hv.astype(np.float64).sum(axis=1).astype(np.float32)

    nb = bl // NF
    fp = fk // 2
    in_maps = []
    for i in range(n_cores):
        xs = x[i * bl:(i + 1) * bl].astype(_NP_F8)  # [bl, f]
        # xt[p, q*(2*bl) + n*(2*NF) + k*NF + b] = xs[n*NF + b, (2q+k)*P + p]
        xt_host = np.ascontiguousarray(
            xs.reshape(nb, NF, fp, 2, P)
            .transpose(4, 2, 0, 3, 1)
            .reshape(P, fk * bl)
        )
        in_maps.append({"xt": xt_host, "w": w_host, "at": at_host})
    return in_maps, rowsum_hv


_NC_CACHE = {}


def _get_nc():
    if "nc" not in _NC_CACHE:
        _NC_CACHE["nc"] = build_nc()
    return _NC_CACHE["nc"]


def run(x, W, classes_hv, trace=False, **spmd_kwargs):
    """Run on 8 NeuronCores; returns (resp_int32, BassKernelResults)."""
    in_maps, rowsum_hv = prep_inputs(x, W, classes_hv)
    nc = _get_nc()
    bk = run_bass_kernel_spmd(
        nc, in_maps, list(range(N_CORES)), trace=trace, **spmd_kwargs
    )
    bl = B // N_CORES
    resp = np.concatenate(
        [r["out"].reshape(bl, C) for r in bk.results], axis=0
    )  # [B, C] f32, integer-valued
    resp = resp + rowsum_hv[None, :]
    return resp.astype(np.int32), bk


def kernel(x, W, classes_hv):
    resp, _ = run(np.asarray(x), np.asarray(W), np.asarray(classes_hv))
    return resp
